# revision 34
# baseline (speedup 1.0000x reference)
"""Multi-head self-attention (B=2, T=2048, D=1024, H=16) on 8 NeuronCores.

Sharding: data-parallel over batch (2) x tensor-parallel over heads
(4 heads per core).  Each core computes, for its batch b and its 4
heads:
  - column-parallel QKV projection (only its heads' rows of w_qkv)
  - causal flash attention for its heads
  - row-parallel out-projection partial (only its heads' columns of
    w_out)
The host sums the 4 partial outputs per batch (the "all-reduce").

Device layouts (per core, fp16 operands / fp32 accumulation):
  xT   [1024, 2048]  x[b] transposed (feature-major)
  wqT/wkT/wvT [1024, 256]   W.T slices for this core's heads
  woT  [256, 1024]   w_out[:, cols].T
  y    [2048, 1024]  fp16 partial output (row-parallel)

Kernel internals: Q,K kept feature-major [d_head, T] so scores are
computed transposed (S_T[tk, tq] = K Q^T) with the k-token dim on
partitions; softmax sums then come for free from a fused ones-column
appended to the token-major V tiles (the PV matmul emits row-sums as
PSUM partition 64).  No P transposes and no max-subtraction needed
(|scores/8| < ~3 so exp is safe).  fp16 matmul operands run the PE at
full rate (fp32 matmuls cost 4x on trn2); PSUM accumulation stays fp32.
Head pairs are row-packed (64-partition row groups) so the d_head=64
score matmuls run two-at-a-time in the 128x128 array.  Causality is
exact: score/exp/PV work below the 128-token diagonal tile is skipped
via shortened access patterns, and the diagonal triangle is zeroed with
affine_select after exp.  The projections of token block j+1 and the
out-projection of earlier blocks are interleaved between attention
chunks as gap fillers so the PE stays busy during exp waits.

Schedule details (perfetto-trace driven):
  - a burst of dummy warm-up matmuls at t~4..11us keeps the PE's HAM
    activity monitor busy during the input DMA so the real matmul
    stream starts at the full 2.4 GHz clock instead of 1.2 GHz;
  - the softmax-normalize chain (reciprocal of the row sums, the
    1/rowsum broadcast matmul and the final multiply) is deferred into
    the NEXT attention stream's filler slots: emitted at a per-group
    boundary it would sit in the PE's in-order queue waiting on the
    DVE chain, stalling the PE long enough for HAM to re-throttle;
  - the two broadcast matmuls are packed in one PE pass via
    tile_position (0,0)/(64,64) quadrants;
  - y is written fp16 (the host sums partials in fp32), halving the
    output DMA in the kernel tail.
"""

import os
import sys

_REPO = "/opt/trn_rl_repo"
if _REPO not in sys.path:
    sys.path.insert(0, _REPO)

import numpy as np

import concourse.bass as bass  # noqa: F401
import concourse.mybir as mybir
import concourse.tile as tile
from concourse import bacc
from concourse.bass_utils import run_bass_kernel_spmd

F32 = mybir.dt.float32
F16 = mybir.dt.float16
AF = mybir.ActivationFunctionType

B = 2
T = 2048
D = 1024
H = 16
DH = 64  # head dim
N_CORES = 8
HPC = H // (N_CORES // B)  # heads per core = 4
E = HPC * DH  # local qkv width = 256
KA = D // 128  # contraction chunks for the projections = 8
NQ = 4  # q blocks of 512
NT = 16  # token tiles of 128
SCALE = 1.0 / np.sqrt(DH)
N_WARM = 18  # dummy matmuls bridging PE-body start to the first x/wk data

_CACHE = {}
LAST_RESULT = None


def _build():
    nc = bacc.Bacc("TRN2", target_bir_lowering=False, debug=False)

    xT = nc.dram_tensor("xT", [D, T], F16, kind="ExternalInput")
    wqT = nc.dram_tensor("wqT", [D, E], F16, kind="ExternalInput")
    wkT = nc.dram_tensor("wkT", [D, E], F16, kind="ExternalInput")
    wvT = nc.dram_tensor("wvT", [D, E], F16, kind="ExternalInput")
    woT = nc.dram_tensor("woT", [E, D], F16, kind="ExternalInput")
    y = nc.dram_tensor("y", [T, D], F16, kind="ExternalOutput")

    xr = xT[:, :].rearrange("(a p) t -> p a t", p=128)  # [128, 8, 2048]
    wqr = wqT[:, :].rearrange("(a p) e -> p a e", p=128)  # [128, 8, 256]
    wkr = wkT[:, :].rearrange("(a p) e -> p a e", p=128)
    wvr = wvT[:, :].rearrange("(a p) e -> p a e", p=128)
    wor = woT[:, :].rearrange("(m p) n -> p m n", p=128)  # [128, 2, 1024]

    with tile.TileContext(nc) as tc:
        with (
            tc.tile_pool(name="persist", bufs=1) as pp,
            tc.tile_pool(name="pt_pool", bufs=8) as ptp,
            tc.tile_pool(name="y_pool", bufs=6) as yp,
            tc.tile_pool(name="r_pool", bufs=6) as rp,
            tc.tile_pool(name="bc_pool", bufs=4) as bcp,
            tc.tile_pool(name="ps_s", bufs=2, space="PSUM") as ps_s,
            tc.tile_pool(name="ps_o", bufs=2, space="PSUM") as ps_o,
            tc.tile_pool(name="ps_b", bufs=2, space="PSUM") as ps_b,
        ):
            # ---- persistent SBUF ----
            wq_sb = pp.tile([128, KA, E], F16, tag="wq")
            wk_sb = pp.tile([128, KA, E], F16, tag="wk")
            wv_sb = pp.tile([128, KA, E], F16, tag="wv")
            wo_sb = pp.tile([128, 2, D], F16, tag="wo")
            xt_sb = pp.tile([128, KA, T], F16, tag="xt")
            qt_sb = [pp.tile([128, T], F16, tag=f"qt{m}", name=f"qt{m}") for m in range(2)]
            kt_sb = [pp.tile([128, T], F16, tag=f"kt{m}", name=f"kt{m}") for m in range(2)]
            at_sb = [pp.tile([128, T], F16, tag=f"at{m}", name=f"at{m}") for m in range(2)]
            # token-major V with a ones column per head, padded to 128
            # weight columns: a full-128-column stationary operand triggers
            # the PE's fast-weight-load path, hiding the PV LDWEIGHTS
            # (~110ns x 320 matmuls) behind the preceding matmuls.
            va_sb = [
                pp.tile([128, HPC, 128], F16, tag="vaug", bufs=NT, name=f"va{t}")
                for t in range(NT)
            ]
            # ones rows at partitions 0 and 64 for the rowsum-reciprocal
            # broadcast matmuls (quadrants (0,0) and (64,64))
            ones65 = pp.tile([DH + 1, DH], F16, tag="ones")
            warm_sb = pp.tile([128, 512], F16, tag="warm")

            # ---- PE warm-up: dummy matmuls with no DMA dependency keep
            # the HAM activity window busy while the inputs stream in, so
            # the real matmuls below start at 2.4 GHz.
            # ---- input DMAs, spread across engine queues by need-order ----
            # descriptor writes serialize per engine (~0.6-0.9us each), so the
            # first-needed transfers (wk, x slice a=0) ride the gpsimd queue,
            # whose post-preamble body starts earliest (~5.7us); the rest of
            # the first x chunk is split across scalar/vector/sync so all of
            # c=0 is in flight by ~9us instead of ~13us.
            nc.gpsimd.memset(warm_sb[:, :], 0.0)
            nc.gpsimd.dma_start(out=wk_sb[:, :, :], in_=wkr)
            for a in (0, 1):
                nc.gpsimd.dma_start(out=xt_sb[:, a, 0:512], in_=xr[:, a, 0:512])
            for a in (2, 3, 4):
                nc.scalar.dma_start(out=xt_sb[:, a, 0:512], in_=xr[:, a, 0:512])
            for a in (5, 6, 7):
                nc.sync.dma_start(out=xt_sb[:, a, 0:512], in_=xr[:, a, 0:512])
            nc.sync.dma_start(out=wq_sb[:, :, :], in_=wqr)
            nc.sync.dma_start(out=wv_sb[:, :, :], in_=wvr)
            for c in range(1, 4):
                sl = slice(512 * c, 512 * (c + 1))
                nc.scalar.dma_start(out=xt_sb[:, :, sl], in_=xr[:, :, sl])
            nc.sync.dma_start(out=wo_sb[:, :, :], in_=wor)

            # preload the ACT exp spline tables (~1.3us) during the DMA
            # window instead of on the first real softmax exp
            warm_e = pp.tile([1, 2], F16, tag="we")
            nc.scalar.activation(
                out=warm_e[:, :], in_=warm_sb[0:1, 0:2], func=AF.Exp, scale=1.0
            )
            for _ in range(N_WARM):
                wps = ps_b.tile([128, 128], F32, tag="b", name="warm_ps")
                nc.tensor.matmul(
                    wps[:, :],
                    lhsT=warm_sb[:, 0:128],
                    rhs=warm_sb[:, 384:512],
                    start=True,
                    stop=True,
                )

            nc.vector.memset(ones65[:, :], 1.0)
            for t in range(NT):
                nc.vector.memset(va_sb[t][:, :, DH : DH + 1], 1.0)
                nc.vector.memset(va_sb[t][:, :, DH + 1 : 128], 0.0)

            def emit_qk_group(c, dst, w, m):
                sl = slice(512 * c, 512 * (c + 1))
                ps = ps_b.tile([128, 512], F32, tag="b", name="ps_proj")
                for a in range(KA):
                    nc.tensor.matmul(
                        ps[:, :],
                        lhsT=w[:, a, 128 * m : 128 * (m + 1)],
                        rhs=xt_sb[:, a, sl],
                        start=(a == 0),
                        stop=(a == KA - 1),
                    )
                nc.vector.tensor_copy(dst[m][:, sl], ps[:, :])

            def emit_v_group(t):
                tsl = slice(128 * t, 128 * (t + 1))
                psv = ps_b.tile([128, E], F32, tag="b", name="ps_v")
                for a in range(KA):
                    nc.tensor.matmul(
                        psv[:, :],
                        lhsT=xt_sb[:, a, tsl],
                        rhs=wv_sb[:, a, :],
                        start=(a == 0),
                        stop=(a == KA - 1),
                    )
                nc.vector.tensor_copy(
                    va_sb[t][:, :, 0:DH],
                    psv[:, :].rearrange("p (h d) -> p h d", h=HPC),
                )

            def proj_fillers(c):
                f = []
                for dst, w in ((kt_sb, wk_sb), (qt_sb, wq_sb)):
                    for m in range(2):
                        f.append(lambda c=c, dst=dst, w=w, m=m: emit_qk_group(c, dst, w, m))
                for t in range(4 * c, 4 * c + 4):
                    f.append(lambda t=t: emit_v_group(t))
                return f

            def emit_y(t, act_copy=False):
                """Full out-projection of token tile t: the two m-halves
                accumulate in PSUM (no separate add).  act_copy routes the
                PSUM evacuation to the scalar engine — used in the kernel
                tail where ACT is idle and the DVE is the critical path."""
                tsl = slice(128 * t, 128 * (t + 1))
                yt = yp.tile([128, D], F16, tag="y", name="y_t")
                for n in range(2):
                    nsl = slice(512 * n, 512 * (n + 1))
                    psy = ps_b.tile([128, 512], F32, tag="b", name="ps_y")
                    for m2 in range(2):
                        nc.tensor.matmul(
                            psy[:, :],
                            lhsT=at_sb[m2][:, tsl],
                            rhs=wo_sb[:, m2, nsl],
                            start=(m2 == 0),
                            stop=(m2 == 1),
                        )
                    if act_copy and n == 0:
                        nc.scalar.copy(out=yt[:, nsl], in_=psy[:, :])
                    else:
                        nc.vector.tensor_copy(yt[:, nsl], psy[:, :])
                nc.sync.dma_start(out=y[tsl, :], in_=yt[:, :])

            def outproj_fillers(j, act_copy=False):
                return [
                    lambda t=t: emit_y(t, act_copy) for t in range(4 * j, 4 * j + 4)
                ]

            def attn_block(j, fillers, reserve=0, tail_norm=False):
                """Attention for q-block j; fillers are interleaved between
                chunks.  The per-group softmax normalization is split: the
                PSUM evacuation + reciprocal run inline (DVE only), while the
                PE-side broadcast matmuls + final multiply are deferred a few
                filler slots into the NEXT stream so the PE's in-order queue
                never waits on the DVE chain.  Returns the finish-closure of
                the last group (the caller decides where to run it)."""
                jsl = slice(512 * j, 512 * (j + 1))
                nch = 2 * (4 * j + 4)
                ci = 0
                fi = 0

                def tick():
                    nonlocal ci, fi
                    ci += 1
                    # +2 front-load: without it the schedule starts with a
                    # filler drought exactly at the block boundary, where the
                    # attention chain is still refilling and the PE would
                    # otherwise idle long enough for HAM to re-throttle
                    want = min(ci * len(fillers) // nch + 2, len(fillers) - reserve)
                    while fi < want:
                        fillers[fi]()
                        fi += 1

                last_fin = None
                for g in range(2):  # head pair group: heads (2g, 2g+1)
                    pso = [
                        ps_o.tile([128, 512], F32, tag="o", name=f"ps_o{hp}")
                        for hp in range(2)
                    ]
                    for i in range(4 * j + 4):  # k tiles of 128
                        r = i - 4 * j
                        col0 = 128 * r if r >= 0 else 0
                        csl = slice(col0, 512)
                        ksl = slice(128 * i, 128 * (i + 1))
                        pss = ps_s.tile([128, 2, 512], F32, tag="s", name="ps_s")
                        for hp in range(2):
                            p0 = 64 * hp
                            nc.tensor.matmul(
                                pss[:, hp, csl],
                                lhsT=kt_sb[g][p0 : p0 + 64, ksl],
                                rhs=qt_sb[g][p0 : p0 + 64, 512 * j + col0 : 512 * (j + 1)],
                                start=True,
                                stop=True,
                            )
                        pt = ptp.tile([128, 2, 512], F16, tag="pt", name="pt")
                        nc.scalar.activation(
                            out=pt[:, :, csl],
                            in_=pss[:, :, csl],
                            func=AF.Exp,
                            scale=float(SCALE),
                        )
                        if r >= 0:  # mask the diagonal 128x128 triangle
                            for hp in range(2):
                                nc.gpsimd.affine_select(
                                    out=pt[:, hp, col0 : col0 + 128],
                                    in_=pt[:, hp, col0 : col0 + 128],
                                    pattern=[[1, 128]],
                                    channel_multiplier=-1,
                                    base=0,
                                    compare_op=mybir.AluOpType.is_ge,
                                    fill=0.0,
                                )
                        for hp in range(2):
                            h = 2 * g + hp
                            nc.tensor.matmul(
                                pso[hp][:, csl],
                                lhsT=va_sb[i][:, h, :],
                                rhs=pt[:, hp, csl],
                                start=(i == 0),
                                stop=(i == 4 * j + 3),
                            )
                        tick()

                    # Evacuate pso to SBUF right away so the PSUM slots free
                    # for the next head pair; the reciprocal chain runs inline
                    # on the DVE, but the PE-side broadcast + final multiply
                    # are deferred (see finish below).
                    # in the tail (last group of the last block) the rowsum
                    # copies and the downcast ride the idle scalar engine so
                    # the DVE only carries au -> reciprocal -> multiplies
                    use_act = tail_norm and g == 1
                    rs = rp.tile([1, 2 * 512], F32, tag="rs", name="rs")
                    aus = []
                    for hp in range(2):
                        if use_act:
                            nc.scalar.copy(
                                out=rs[:, 512 * hp : 512 * (hp + 1)],
                                in_=pso[hp][DH : DH + 1, :],
                            )
                        else:
                            nc.vector.tensor_copy(
                                rs[:, 512 * hp : 512 * (hp + 1)],
                                pso[hp][DH : DH + 1, :],
                            )
                        au = bcp.tile([DH, 512], F32, tag="bc", name="au")
                        nc.vector.tensor_copy(au[:, :], pso[hp][0:DH, :])
                        aus.append(au)
                    rec = rp.tile([1, 2 * 512], F32, tag="rec", name="rec")
                    nc.vector.reciprocal_approx_fast(out=rec[:, :], in_=rs[:, :])
                    rec16 = rp.tile([1, 2 * 512], F16, tag="rec16", name="rec16")
                    if use_act:
                        nc.scalar.copy(out=rec16[:, :], in_=rec[:, :])
                    else:
                        nc.vector.tensor_copy(rec16[:, :], rec[:, :])

                    def finish(g=g, aus=aus, rec16=rec16):
                        for hp in range(2):
                            p0 = 64 * hp
                            psb = ps_b.tile([DH, 512], F32, tag="b", name="ps_bc")
                            nc.tensor.matmul(
                                psb[:, :],
                                lhsT=ones65[0:1, :],
                                rhs=rec16[:, 512 * hp : 512 * (hp + 1)],
                                start=True,
                                stop=True,
                            )
                            nc.vector.tensor_mul(
                                at_sb[g][p0 : p0 + 64, jsl], aus[hp][:, :], psb[:, :]
                            )

                    if g == 0:
                        # defer into this block's second stream, a few slots
                        # in so the DVE chain has finished by then — but
                        # always inside the consumed zone (not the reserved
                        # tail), so its muls don't land on the tail DVE path
                        pos = min(fi + 3, max(fi, len(fillers) - reserve))
                        fillers.insert(pos, finish)
                    else:
                        last_fin = finish
                    tick()
                    tick()
                while fi < len(fillers) - reserve:
                    fillers[fi]()
                    fi += 1
                return last_fin, fillers[len(fillers) - reserve :] if reserve else []

            # software pipeline: attention j overlaps projections of block j+1;
            # out-projections of blocks 0..2 fill the last attention block,
            # where the exp stream leaves the most PE slack.  Block 3's own
            # out-projection runs in the tail with PSUM-accumulated m-halves
            # (the "all-reduce add" is free) and ACT/DVE-split evacuations.
            for fn in proj_fillers(0):
                fn()
            fin_prev = None
            for j in range(NQ):
                fillers = []
                if fin_prev is not None:
                    fillers.append(fin_prev)
                if j + 1 < NQ:
                    fillers += proj_fillers(j + 1)
                    fin_prev, _ = attn_block(j, fillers)
                else:
                    fillers += outproj_fillers(0)
                    fillers += outproj_fillers(1)
                    fillers += outproj_fillers(2, act_copy=True)
                    fin_last, rest = attn_block(
                        j, fillers, reserve=3, tail_norm=True
                    )
                    # tail: one reserved filler covers the PE while the ACT/
                    # DVE normalize chain runs; the final multiplies come
                    # right after the reciprocal in the DVE queue (before the
                    # remaining fillers' copies, which ride ACT)
                    rest[0]()
                    fin_last()
                    for fn in rest[1:]:
                        fn()
            for t in range(4 * (NQ - 1), 4 * NQ):
                emit_y(t, act_copy=True)

    nc.compile()
    return nc


def _get_nc():
    if "nc" not in _CACHE:
        _CACHE["nc"] = _build()
    return _CACHE["nc"]


def kernel(x, w_qkv, w_out):
    global LAST_RESULT
    x = np.asarray(x, dtype=np.float32)
    w_qkv = np.asarray(w_qkv, dtype=np.float32)
    w_out = np.asarray(w_out, dtype=np.float32)

    nc = _get_nc()

    in_maps = []
    for core in range(N_CORES):
        b = core // (N_CORES // B)
        hg = core % (N_CORES // B)
        e0 = hg * E  # first feature of this core's heads
        in_maps.append(
            {
                "xT": np.ascontiguousarray(x[b].T).astype(np.float16),
                "wqT": np.ascontiguousarray(w_qkv[e0 : e0 + E, :].T).astype(np.float16),
                "wkT": np.ascontiguousarray(
                    w_qkv[D + e0 : D + e0 + E, :].T
                ).astype(np.float16),
                "wvT": np.ascontiguousarray(
                    w_qkv[2 * D + e0 : 2 * D + e0 + E, :].T
                ).astype(np.float16),
                "woT": np.ascontiguousarray(w_out[:, e0 : e0 + E].T).astype(np.float16),
            }
        )

    trace = bool(os.environ.get("BASS_TRACE"))
    try:
        res = run_bass_kernel_spmd(
            nc, in_maps, core_ids=list(range(N_CORES)), trace=trace
        )
    except Exception:
        if not trace:
            raise
        # tracing infrastructure unavailable in this environment; the
        # compute path does not need it
        os.environ["BASS_NEVER_TRACE"] = "1"
        res = run_bass_kernel_spmd(
            nc, in_maps, core_ids=list(range(N_CORES)), trace=False
        )
    LAST_RESULT = res

    out = np.zeros((B, T, D), dtype=np.float32)
    for core in range(N_CORES):
        b = core // (N_CORES // B)
        out[b] += res.results[core]["y"].astype(np.float32)
    return out
